# revision 3
# baseline (speedup 1.0000x reference)
"""Trainium2 Bass kernel v2c for nn_CombinedTargetIOULoss (B=64, K=17, H=W=64).

Layout: partition = (pair-half, hx), free = (ch, hy); inputs stream in as
bf16 via gpsimd SWDGE cast-DMA (~257 GB/s, the fastest measured path).
GPSIMD does NO elementwise work (its SBUF port is shared with the DVE and
contention inflates DVE ops 2-4x) - it only issues DMA.

Compute split:
 - ACT: all abs/square (1x spline), psum evacuation.
 - DVE: everything else in bf16 (tensor_tensor = 2x), with x|y PAIRED ops
   on concatenated tiles to halve instruction overhead, and ONE
   reciprocal_approx_fast over (ue | ac4) with imm2=5.0 (computes ~4/x;
   4/ue feeds q1' = inter*4/ue = 4*q1 -> host divides by 4;
   4/ac4 = 1/ace exactly -> q2' = q2). eps terms dropped (bf16 inputs
   cannot produce exact-zero areas for gaussian data).
 - PE: one-hot matmuls accumulate per-(b,k)-per-hy sums across iterations
   into 3 psum groups (q1', q2', dsq); hy summed on host.

Raw bass; standalone monotone waits.
"""

import sys

sys.path.insert(0, "/opt/trn_rl_repo")

import numpy as np

import concourse.bass as bass
from concourse import mybir
from concourse.alu_op_type import AluOpType as Alu
from concourse.bass_utils import run_bass_kernel_spmd
from concourse.dve_ops import (
    RECIP_APPROX_FAST_CONSTS as RAF_CONSTS,
    RECIPROCAL_APPROX_FAST as RAF_OP,
)

F32 = mybir.dt.float32
BF16 = mybir.dt.bfloat16
AF = mybir.ActivationFunctionType

B, K, H, W = 64, 17, 64, 64
C = 3 * K
P = H * W
N_CORES = 8
B_LOC = B // N_CORES
N_PAIR = B_LOC // 2
J = 64
MIDF = K * J            # 1088
MID2 = 2 * MIDF         # 2176 (x|y interleaved as (k, c2, hy))
SPLITS = [(0, 6), (6, 6), (12, 5)]

N_DVE = 14
N_ACT = 6
N_PE = 9                # 3 splits x (dsq, q1, q2)
GP0 = 3 * N_PAIR        # one-hot memsets


class _Waiter:
    def __init__(self):
        self.seen = {}

    def wait(self, eng, sem, val):
        if val <= 0:
            return
        key = (id(eng), sem.name if hasattr(sem, "name") else id(sem))
        if self.seen.get(key, -1) >= val:
            return
        self.seen[key] = val
        eng.wait_ge(sem, val)


def _build_body(nc, o_ext, t_ext, p_ext):
    sb = lambda name, shape, dt: nc.alloc_sbuf_tensor(name, shape, dt).ap()

    to = [sb(f"to{s}", [128, C * J], BF16) for s in range(2)]
    tt = [sb(f"tt{s}", [128, C * J], BF16) for s in range(2)]
    m2 = {}
    for nm in "exy apq agh dxy sxy uvm uvp uac rc".split():
        m2[nm] = sb(nm, [128, MID2], BF16)
    m1 = {}
    for nm in "d t1 t2 s it4 inter inter2 q1 q2 dsq".split():
        m1[nm] = sb(nm, [128, MIDF], BF16)
    wts = [sb(f"w{j}", [128, B_LOC], BF16) for j in range(N_PAIR)]
    osb = sb("osb", [B_LOC, 2 * K * J], F32)
    dmy = sb("dmy", [128, 4], F32)
    ps = {}
    for qi in range(2):  # 0: q1+q2, 1: dsq
        for si, (k0, n) in enumerate(SPLITS):
            ps[qi, si] = nc.alloc_psum_tensor(
                f"ps{qi}{si}", [B_LOC, n * J], F32
            ).ap()

    dma_o = nc.alloc_semaphore("dma_o")
    dma_t = nc.alloc_semaphore("dma_t")
    dma_out = nc.alloc_semaphore("dma_out")
    act_c = nc.alloc_semaphore("act_c")
    dve_c = nc.alloc_semaphore("dve_c")
    pe_c = nc.alloc_semaphore("pe_c")
    gp_c = nc.alloc_semaphore("gp_c")
    wt = _Waiter()

    def comp0(T):
        return T.rearrange("p (k c hy) -> p k c hy", k=K, c=3, hy=J)[:, :, 0]

    def comp12(T):
        return T.rearrange("p (k c hy) -> p k c hy", k=K, c=3, hy=J)[:, :, 1:3]

    def xhalf(T2):  # x-component of an interleaved (k, 2, hy) tile
        return T2.rearrange("p (k c hy) -> p k c hy", k=K, c=2, hy=J)[:, :, 0]

    def yhalf(T2):
        return T2.rearrange("p (k c hy) -> p k c hy", k=K, c=2, hy=J)[:, :, 1]

    # --- warmup ACT tables ---
    nc.scalar.activation(dmy[:, 0:1], dmy[:, 3:4], AF.Abs)
    nc.scalar.activation(dmy[:, 1:2], dmy[:, 3:4], AF.Square)
    nc.scalar.activation(dmy[:, 2:3], dmy[:, 3:4], AF.Copy, bias=0.0, scale=1.0)

    def act(out, in_, func, **kw):
        nc.scalar.activation(out, in_, func, **kw).then_inc(act_c, 1)

    def dve_tt(out, a, b, op):
        nc.vector.tensor_tensor(out, a, b, op).then_inc(dve_c, 1)

    def issue_dma(j):
        sl = j % 2
        if j >= 2:
            wt.wait(nc.gpsimd, act_c, N_ACT * (j - 2) + 2)  # apq/agh read inputs
            wt.wait(nc.gpsimd, dve_c, N_DVE * (j - 2) + 10)  # exy/d read inputs
        for pi in range(2):
            nc.gpsimd.dma_start(
                out=to[sl][64 * pi : 64 * pi + 64, :].rearrange(
                    "p (ch hy) -> p ch hy", ch=C, hy=J
                ),
                in_=o_ext[2 * j + pi].rearrange("ch hx hy -> hx ch hy"),
            ).then_inc(dma_o, 16)
            nc.gpsimd.dma_start(
                out=tt[sl][64 * pi : 64 * pi + 64, :].rearrange(
                    "p (ch hy) -> p ch hy", ch=C, hy=J
                ),
                in_=t_ext[2 * j + pi].rearrange("ch hx hy -> hx ch hy"),
            ).then_inc(dma_t, 16)

    issue_dma(0)
    # --- one-hot stationary weights ---
    for jw in range(N_PAIR):
        nc.gpsimd.memset(wts[jw][:], 0.0).then_inc(gp_c, 1)
        nc.gpsimd.memset(wts[jw][0:64, 2 * jw : 2 * jw + 1], 1.0).then_inc(
            gp_c, 1
        )
        nc.gpsimd.memset(wts[jw][64:128, 2 * jw + 1 : 2 * jw + 2], 1.0).then_inc(
            gp_c, 1
        )

    for j in range(N_PAIR):
        sl = j % 2
        dve0 = N_DVE * j
        act0 = N_ACT * j
        pe0 = N_PE * j

        if j + 1 < N_PAIR:
            issue_dma(j + 1)

        # --- ACT: paired abs, scalar scales, square ---
        if j >= 1:
            wt.wait(nc.scalar, dve_c, N_DVE * (j - 1) + 4)  # apq/agh free
        wt.wait(nc.scalar, dma_o, 32 * j + 32)
        act(m2["apq"][:], comp12(to[sl]), AF.Abs)                           # 1
        wt.wait(nc.scalar, dma_t, 32 * j + 32)
        act(m2["agh"][:], comp12(tt[sl]), AF.Abs)                           # 2
        if j >= 1:
            wt.wait(nc.scalar, dve_c, N_DVE * (j - 1) + 6)  # dxy free
        wt.wait(nc.scalar, dve_c, dve0 + 1)
        act(m2["dxy"][:], m2["exy"][:], AF.Abs)                             # 3
        if j >= 1:
            wt.wait(nc.scalar, dve_c, N_DVE * (j - 1) + 11)  # inter read by ue
        wt.wait(nc.scalar, dve_c, dve0 + 7)
        act(m1["inter"][:], m1["it4"][:], AF.Copy, bias=0.0, scale=0.25)    # 4
        if j >= 1:
            wt.wait(nc.scalar, dve_c, N_DVE * (j - 1) + 13)  # inter2 read by q1
        act(m1["inter2"][:], m1["it4"][:], AF.Copy, bias=0.0, scale=0.0625)  # 5
        if j >= 1:
            wt.wait(nc.scalar, pe_c, N_PE * (j - 1) + 3)    # dsq matmuls j-1
        wt.wait(nc.scalar, dve_c, dve0 + 10)
        act(m1["dsq"][:], m1["d"][:], AF.Square)                            # 6

        # --- DVE: bf16 box algebra (x|y paired) ---
        if j >= 1:
            wt.wait(nc.vector, act_c, N_ACT * (j - 1) + 3)  # exy free
        wt.wait(nc.vector, dma_o, 32 * j + 32)
        wt.wait(nc.vector, dma_t, 32 * j + 32)
        dve_tt(m2["exy"][:], comp12(to[sl]), comp12(tt[sl]), Alu.subtract)  # 1
        wt.wait(nc.vector, act_c, act0 + 2)
        dve_tt(m2["sxy"][:], m2["apq"][:], m2["agh"][:], Alu.add)           # 2
        dve_tt(m1["t1"][:], xhalf(m2["apq"]), yhalf(m2["apq"]), Alu.mult)   # 3
        dve_tt(m1["t2"][:], xhalf(m2["agh"]), yhalf(m2["agh"]), Alu.mult)   # 4
        wt.wait(nc.vector, act_c, act0 + 3)
        dve_tt(m2["uvm"][:], m2["sxy"][:], m2["dxy"][:], Alu.subtract)      # 5
        dve_tt(m2["uvp"][:], m2["sxy"][:], m2["dxy"][:], Alu.add)           # 6
        dve_tt(m1["it4"][:], xhalf(m2["uvm"]), yhalf(m2["uvm"]), Alu.mult)  # 7
        dve_tt(m2["uac"][:, MIDF:], xhalf(m2["uvp"]), yhalf(m2["uvp"]),
               Alu.mult)                                                    # 8 ac4
        dve_tt(m1["s"][:], m1["t1"][:], m1["t2"][:], Alu.add)               # 9
        if j >= 1:
            wt.wait(nc.vector, act_c, N_ACT * (j - 1) + 6)  # d free (dsq read)
        dve_tt(m1["d"][:], comp0(to[sl]), comp0(tt[sl]), Alu.subtract)      # 10
        wt.wait(nc.vector, act_c, act0 + 4)
        dve_tt(m2["uac"][:, :MIDF], m1["s"][:], m1["inter"][:],
               Alu.subtract)                                                # 11 ue
        nc.vector._custom_dve(
            RAF_OP, out=m2["rc"][:], in0=m2["uac"][:],
            s0=RAF_CONSTS["s0"], s1=RAF_CONSTS["s1"], imm2=5.0,
        ).then_inc(dve_c, 1)                                                # 12
        if j >= 1:
            wt.wait(nc.vector, pe_c, N_PE * (j - 1) + 6)    # WAR q1
        wt.wait(nc.vector, act_c, act0 + 5)
        dve_tt(m1["q1"][:], m1["inter2"][:], m2["rc"][:, :MIDF], Alu.mult)  # 13
        if j >= 1:
            wt.wait(nc.vector, pe_c, N_PE * (j - 1) + 9)    # WAR q2
        dve_tt(m1["q2"][:], m2["uac"][:, :MIDF], m2["rc"][:, MIDF:],
               Alu.mult)                                                    # 14

        # --- PE ---
        if j == 0:
            wt.wait(nc.tensor, gp_c, GP0)
        qv = lambda nm, k0, n: m1[nm].rearrange(
            "p (k hy) -> p k hy", k=K, hy=J
        )[:, k0 : k0 + n]

        wt.wait(nc.tensor, act_c, act0 + 6)
        for si, (k0, n) in enumerate(SPLITS):
            nc.tensor.matmul(
                ps[1, si][:], wts[j][:], qv("dsq", k0, n),
                start=(j == 0), stop=(j == N_PAIR - 1),
            ).then_inc(pe_c, 1)
        wt.wait(nc.tensor, dve_c, dve0 + 13)
        for si, (k0, n) in enumerate(SPLITS):
            nc.tensor.matmul(
                ps[0, si][:], wts[j][:], qv("q1", k0, n),
                start=(j == 0), stop=False,
            ).then_inc(pe_c, 1)
        wt.wait(nc.tensor, dve_c, dve0 + 14)
        for si, (k0, n) in enumerate(SPLITS):
            nc.tensor.matmul(
                ps[0, si][:], wts[j][:], qv("q2", k0, n),
                start=False, stop=(j == N_PAIR - 1),
            ).then_inc(pe_c, 1)

    # --- epilogue: dsq psums evacuate early, then q psums ---
    wt.wait(nc.scalar, pe_c, N_PE * (N_PAIR - 1) + 3)
    nev = 0
    for si, (k0, n) in enumerate(SPLITS):
        act(osb[:, K * J + k0 * J : K * J + (k0 + n) * J],
            ps[1, si][:], AF.Copy, bias=0.0, scale=1.0)
        nev += 1
    wt.wait(nc.sync, act_c, N_ACT * N_PAIR + nev)
    nc.sync.dma_start(out=p_ext[:, K * J :], in_=osb[:, K * J :]).then_inc(
        dma_out, 16
    )
    wt.wait(nc.scalar, pe_c, N_PE * N_PAIR)
    for si, (k0, n) in enumerate(SPLITS):
        act(osb[:, k0 * J : (k0 + n) * J], ps[0, si][:],
            AF.Copy, bias=0.0, scale=1.0)
        nev += 1
    wt.wait(nc.sync, act_c, N_ACT * N_PAIR + nev)
    nc.sync.dma_start(out=p_ext[:, : K * J], in_=osb[:, : K * J]).then_inc(
        dma_out, 16
    )
    nc.sync.wait_ge(dma_out, 32)


def build_nc():
    nc = bass.Bass()
    o_ext = nc.declare_dram_parameter("output", [B_LOC, C, H, W], F32, isOutput=False)
    t_ext = nc.declare_dram_parameter("target", [B_LOC, C, H, W], F32, isOutput=False)
    p_ext = nc.declare_dram_parameter("partials", [B_LOC, 2 * K * J], F32,
                                      isOutput=True)
    _build_body(nc, o_ext, t_ext, p_ext)
    mybir.codegen_inst_isa_subclasses(nc)
    return nc


_NC = None


def _get_nc():
    global _NC
    if _NC is None:
        _NC = build_nc()
    return _NC


def _combine(parts, target_weights):
    """parts: [8 cores, 8, 3*K*64] f32 -> scalar loss.

    osb col layout: [qi(2: 0=q1+q2, 1=dsq), k(17), hy(64)]
    """
    arr = np.asarray(parts, np.float64).reshape(B, 2, K, 64).sum(axis=3)
    sqs = arr[:, 0]
    ssd = arr[:, 1]

    tw = np.asarray(target_weights, np.float64)
    twnz = (tw != 0).astype(np.float64)
    num = ((2.0 * P - sqs) * twnz).sum(axis=0)
    den = np.maximum((P * twnz).sum(axis=0), 1.0)
    giou_joint = num / den
    mse = 0.5 * (tw**2 * ssd).sum(axis=0) / (B * P)
    return np.float32(np.sum(mse + giou_joint) / K)


def kernel(output, target, target_weights):
    output = np.ascontiguousarray(np.asarray(output), dtype=np.float32)
    target = np.ascontiguousarray(np.asarray(target), dtype=np.float32)
    nc = _get_nc()
    in_maps = [
        {
            "output": output[i * B_LOC : (i + 1) * B_LOC],
            "target": target[i * B_LOC : (i + 1) * B_LOC],
        }
        for i in range(N_CORES)
    ]
    res = run_bass_kernel_spmd(nc, in_maps, list(range(N_CORES)))
    parts = np.stack([res.results[i]["partials"] for i in range(N_CORES)])
    return np.asarray(_combine(parts, target_weights), dtype=np.float32)


# revision 5
# speedup vs baseline: 1.1106x; 1.1106x over previous
"""Trainium2 Bass kernel v2c for nn_CombinedTargetIOULoss (B=64, K=17, H=W=64).

Layout: partition = (pair-half, hx), free = (ch, hy); inputs stream in as
bf16 via gpsimd SWDGE cast-DMA (~257 GB/s, the fastest measured path).
GPSIMD does NO elementwise work (its SBUF port is shared with the DVE and
contention inflates DVE ops 2-4x) - it only issues DMA.

Compute split:
 - ACT: all abs/square (1x spline), psum evacuation.
 - DVE: everything else in bf16 (tensor_tensor = 2x), with x|y PAIRED ops
   on concatenated tiles to halve instruction overhead, and ONE
   reciprocal_approx_fast over (ue | ac4) with imm2=5.0 (computes ~4/x;
   4/ue feeds q1' = inter*4/ue = 4*q1 -> host divides by 4;
   4/ac4 = 1/ace exactly -> q2' = q2). eps terms dropped (bf16 inputs
   cannot produce exact-zero areas for gaussian data).
 - PE: one-hot matmuls accumulate per-(b,k)-per-hy sums across iterations
   into 3 psum groups (q1', q2', dsq); hy summed on host.

Raw bass; standalone monotone waits.
"""

import sys

sys.path.insert(0, "/opt/trn_rl_repo")

import numpy as np

import concourse.bass as bass
from concourse import mybir
from concourse.alu_op_type import AluOpType as Alu
from concourse.bass_utils import run_bass_kernel_spmd
from concourse.dve_ops import (
    RECIP_APPROX_FAST_CONSTS as RAF_CONSTS,
    RECIPROCAL_APPROX_FAST as RAF_OP,
)

F32 = mybir.dt.float32
BF16 = mybir.dt.bfloat16
AF = mybir.ActivationFunctionType

B, K, H, W = 64, 17, 64, 64
C = 3 * K
P = H * W
N_CORES = 8
B_LOC = B // N_CORES
N_PAIR = B_LOC // 2
J = 64
MIDF = K * J            # 1088
MID2 = 2 * MIDF         # 2176 (x|y interleaved as (k, c2, hy))
SPLITS = [(0, 6), (6, 6), (12, 5)]

N_DVE = 14
N_ACT = 6
N_PE = 9                # 3 splits x (dsq, q1, q2)
GP0 = 3 * N_PAIR        # one-hot memsets


class _Waiter:
    def __init__(self):
        self.seen = {}

    def wait(self, eng, sem, val):
        if val <= 0:
            return
        key = (id(eng), sem.name if hasattr(sem, "name") else id(sem))
        if self.seen.get(key, -1) >= val:
            return
        self.seen[key] = val
        eng.wait_ge(sem, val)


def _build_body(nc, o_ext, t_ext, p_ext):
    sb = lambda name, shape, dt: nc.alloc_sbuf_tensor(name, shape, dt).ap()

    to = [sb(f"to{s}", [128, C * J], BF16) for s in range(2)]
    tt = [sb(f"tt{s}", [128, C * J], BF16) for s in range(2)]
    m2 = {}
    for nm in "exy apq agh dxy sxy uvm uvp uac rc".split():
        m2[nm] = sb(nm, [128, MID2], BF16)
    m1 = {}
    for nm in "d t1 t2 s it4 inter inter2 q1 q2 dsq".split():
        m1[nm] = sb(nm, [128, MIDF], BF16)
    wts = [sb(f"w{j}", [128, B_LOC], BF16) for j in range(N_PAIR)]
    osb = sb("osb", [B_LOC, 2 * K * J], F32)
    dmy = sb("dmy", [128, 4], F32)
    ps = {}
    for qi in range(2):  # 0: q1+q2, 1: dsq
        for si, (k0, n) in enumerate(SPLITS):
            ps[qi, si] = nc.alloc_psum_tensor(
                f"ps{qi}{si}", [B_LOC, n * J], F32
            ).ap()

    dma_o = nc.alloc_semaphore("dma_o")
    dma_t = nc.alloc_semaphore("dma_t")
    dma_out = nc.alloc_semaphore("dma_out")
    act_c = nc.alloc_semaphore("act_c")
    dve_c = nc.alloc_semaphore("dve_c")
    pe_c = nc.alloc_semaphore("pe_c")
    gp_c = nc.alloc_semaphore("gp_c")
    wt = _Waiter()

    def comp0(T):
        return T.rearrange("p (k c hy) -> p k c hy", k=K, c=3, hy=J)[:, :, 0]

    def comp12(T):
        return T.rearrange("p (k c hy) -> p k c hy", k=K, c=3, hy=J)[:, :, 1:3]

    def xhalf(T2):  # x-component of an interleaved (k, 2, hy) tile
        return T2.rearrange("p (k c hy) -> p k c hy", k=K, c=2, hy=J)[:, :, 0]

    def yhalf(T2):
        return T2.rearrange("p (k c hy) -> p k c hy", k=K, c=2, hy=J)[:, :, 1]

    # --- warmup ACT tables ---
    nc.scalar.activation(dmy[:, 0:1], dmy[:, 3:4], AF.Abs)
    nc.scalar.activation(dmy[:, 1:2], dmy[:, 3:4], AF.Square)
    nc.scalar.activation(dmy[:, 2:3], dmy[:, 3:4], AF.Copy, bias=0.0, scale=1.0)

    def act(out, in_, func, **kw):
        nc.scalar.activation(out, in_, func, **kw).then_inc(act_c, 1)

    def dve_tt(out, a, b, op):
        nc.vector.tensor_tensor(out, a, b, op).then_inc(dve_c, 1)

    def issue_dma(j):
        sl = j % 2
        if j >= 2:
            wt.wait(nc.gpsimd, act_c, N_ACT * (j - 2) + 2)  # apq/agh read inputs
            wt.wait(nc.gpsimd, dve_c, N_DVE * (j - 2) + 2)  # exy/d read inputs
        for pi in range(2):
            nc.gpsimd.dma_start(
                out=to[sl][64 * pi : 64 * pi + 64, :].rearrange(
                    "p (ch hy) -> p ch hy", ch=C, hy=J
                ),
                in_=o_ext[2 * j + pi].rearrange("ch hx hy -> hx ch hy"),
            ).then_inc(dma_o, 16)
            nc.gpsimd.dma_start(
                out=tt[sl][64 * pi : 64 * pi + 64, :].rearrange(
                    "p (ch hy) -> p ch hy", ch=C, hy=J
                ),
                in_=t_ext[2 * j + pi].rearrange("ch hx hy -> hx ch hy"),
            ).then_inc(dma_t, 16)

    issue_dma(0)
    # --- one-hot stationary weights ---
    for jw in range(N_PAIR):
        nc.gpsimd.memset(wts[jw][:], 0.0).then_inc(gp_c, 1)
        nc.gpsimd.memset(wts[jw][0:64, 2 * jw : 2 * jw + 1], 1.0).then_inc(
            gp_c, 1
        )
        nc.gpsimd.memset(wts[jw][64:128, 2 * jw + 1 : 2 * jw + 2], 1.0).then_inc(
            gp_c, 1
        )

    for j in range(N_PAIR):
        sl = j % 2
        dve0 = N_DVE * j
        act0 = N_ACT * j
        pe0 = N_PE * j

        if j + 1 < N_PAIR:
            issue_dma(j + 1)

        # --- ACT: paired abs, scalar scales, square ---
        if j >= 1:
            wt.wait(nc.scalar, dve_c, N_DVE * (j - 1) + 5)  # apq/agh free
        wt.wait(nc.scalar, dma_o, 32 * j + 32)
        act(m2["apq"][:], comp12(to[sl]), AF.Abs)                           # 1
        wt.wait(nc.scalar, dma_t, 32 * j + 32)
        act(m2["agh"][:], comp12(tt[sl]), AF.Abs)                           # 2
        if j >= 1:
            wt.wait(nc.scalar, dve_c, N_DVE * (j - 1) + 8)  # dxy free
        wt.wait(nc.scalar, dve_c, dve0 + 1)
        act(m2["dxy"][:], m2["exy"][:], AF.Abs)                             # 3
        if j >= 1:
            wt.wait(nc.scalar, dve_c, N_DVE * (j - 1) + 11)  # inter read by ue
        wt.wait(nc.scalar, dve_c, dve0 + 9)
        act(m1["inter"][:], m1["it4"][:], AF.Copy, bias=0.0, scale=0.25)    # 4
        if j >= 1:
            wt.wait(nc.scalar, dve_c, N_DVE * (j - 1) + 13)  # inter2 read by q1
        act(m1["inter2"][:], m1["it4"][:], AF.Copy, bias=0.0, scale=0.0625)  # 5
        if j >= 1:
            wt.wait(nc.scalar, pe_c, N_PE * (j - 1) + 3)    # dsq matmuls j-1
        wt.wait(nc.scalar, dve_c, dve0 + 2)
        act(m1["dsq"][:], m1["d"][:], AF.Square)                            # 6

        # --- DVE: bf16 box algebra (x|y paired) ---
        if j >= 1:
            wt.wait(nc.vector, act_c, N_ACT * (j - 1) + 3)  # exy free (dxy done)
        wt.wait(nc.vector, dma_o, 32 * j + 32)
        wt.wait(nc.vector, dma_t, 32 * j + 32)
        dve_tt(m2["exy"][:], comp12(to[sl]), comp12(tt[sl]), Alu.subtract)  # 1
        if j >= 1:
            wt.wait(nc.vector, act_c, N_ACT * (j - 1) + 6)  # d free (dsq done)
        dve_tt(m1["d"][:], comp0(to[sl]), comp0(tt[sl]), Alu.subtract)      # 2
        wt.wait(nc.vector, act_c, act0 + 2)
        dve_tt(m2["sxy"][:], m2["apq"][:], m2["agh"][:], Alu.add)           # 3
        dve_tt(m1["t1"][:], xhalf(m2["apq"]), yhalf(m2["apq"]), Alu.mult)   # 4
        dve_tt(m1["t2"][:], xhalf(m2["agh"]), yhalf(m2["agh"]), Alu.mult)   # 5
        dve_tt(m1["s"][:], m1["t1"][:], m1["t2"][:], Alu.add)               # 6
        wt.wait(nc.vector, act_c, act0 + 3)
        dve_tt(m2["uvm"][:], m2["sxy"][:], m2["dxy"][:], Alu.subtract)      # 7
        dve_tt(m2["uvp"][:], m2["sxy"][:], m2["dxy"][:], Alu.add)           # 8
        dve_tt(m1["it4"][:], xhalf(m2["uvm"]), yhalf(m2["uvm"]), Alu.mult)  # 9
        dve_tt(m2["uac"][:, MIDF:], xhalf(m2["uvp"]), yhalf(m2["uvp"]),
               Alu.mult)                                                    # 10 ac4
        wt.wait(nc.vector, act_c, act0 + 4)
        dve_tt(m2["uac"][:, :MIDF], m1["s"][:], m1["inter"][:],
               Alu.subtract)                                                # 11 ue
        nc.vector._custom_dve(
            RAF_OP, out=m2["rc"][:], in0=m2["uac"][:],
            s0=RAF_CONSTS["s0"], s1=RAF_CONSTS["s1"], imm2=5.0,
        ).then_inc(dve_c, 1)                                                # 12
        if j >= 1:
            wt.wait(nc.vector, pe_c, N_PE * (j - 1) + 6)    # WAR q1
        wt.wait(nc.vector, act_c, act0 + 5)
        dve_tt(m1["q1"][:], m1["inter2"][:], m2["rc"][:, :MIDF], Alu.mult)  # 13
        if j >= 1:
            wt.wait(nc.vector, pe_c, N_PE * (j - 1) + 9)    # WAR q2
        dve_tt(m1["q2"][:], m2["uac"][:, :MIDF], m2["rc"][:, MIDF:],
               Alu.mult)                                                    # 14

        # --- PE ---
        if j == 0:
            wt.wait(nc.tensor, gp_c, GP0)
        qv = lambda nm, k0, n: m1[nm].rearrange(
            "p (k hy) -> p k hy", k=K, hy=J
        )[:, k0 : k0 + n]

        wt.wait(nc.tensor, act_c, act0 + 6)
        for si, (k0, n) in enumerate(SPLITS):
            nc.tensor.matmul(
                ps[1, si][:], wts[j][:], qv("dsq", k0, n),
                start=(j == 0), stop=(j == N_PAIR - 1),
            ).then_inc(pe_c, 1)
        wt.wait(nc.tensor, dve_c, dve0 + 13)
        for si, (k0, n) in enumerate(SPLITS):
            nc.tensor.matmul(
                ps[0, si][:], wts[j][:], qv("q1", k0, n),
                start=(j == 0), stop=False,
            ).then_inc(pe_c, 1)
        wt.wait(nc.tensor, dve_c, dve0 + 14)
        for si, (k0, n) in enumerate(SPLITS):
            nc.tensor.matmul(
                ps[0, si][:], wts[j][:], qv("q2", k0, n),
                start=False, stop=(j == N_PAIR - 1),
            ).then_inc(pe_c, 1)

    # --- epilogue: dsq psums evacuate early, then q psums ---
    wt.wait(nc.scalar, pe_c, N_PE * (N_PAIR - 1) + 3)
    nev = 0
    for si, (k0, n) in enumerate(SPLITS):
        act(osb[:, K * J + k0 * J : K * J + (k0 + n) * J],
            ps[1, si][:], AF.Copy, bias=0.0, scale=1.0)
        nev += 1
    wt.wait(nc.sync, act_c, N_ACT * N_PAIR + nev)
    nc.sync.dma_start(out=p_ext[:, K * J :], in_=osb[:, K * J :]).then_inc(
        dma_out, 16
    )
    for si, (k0, n) in enumerate(SPLITS):
        wt.wait(nc.scalar, pe_c, N_PE * (N_PAIR - 1) + 7 + si)
        act(osb[:, k0 * J : (k0 + n) * J], ps[0, si][:],
            AF.Copy, bias=0.0, scale=1.0)
        nev += 1
    wt.wait(nc.sync, act_c, N_ACT * N_PAIR + nev)
    nc.sync.dma_start(out=p_ext[:, : K * J], in_=osb[:, : K * J]).then_inc(
        dma_out, 16
    )
    nc.sync.wait_ge(dma_out, 32)


def build_nc():
    nc = bass.Bass()
    o_ext = nc.declare_dram_parameter("output", [B_LOC, C, H, W], F32, isOutput=False)
    t_ext = nc.declare_dram_parameter("target", [B_LOC, C, H, W], F32, isOutput=False)
    p_ext = nc.declare_dram_parameter("partials", [B_LOC, 2 * K * J], F32,
                                      isOutput=True)
    _build_body(nc, o_ext, t_ext, p_ext)
    mybir.codegen_inst_isa_subclasses(nc)
    return nc


_NC = None


def _get_nc():
    global _NC
    if _NC is None:
        _NC = build_nc()
    return _NC


def _combine(parts, target_weights):
    """parts: [8 cores, 8, 3*K*64] f32 -> scalar loss.

    osb col layout: [qi(2: 0=q1+q2, 1=dsq), k(17), hy(64)]
    """
    arr = np.asarray(parts, np.float64).reshape(B, 2, K, 64).sum(axis=3)
    sqs = arr[:, 0]
    ssd = arr[:, 1]

    tw = np.asarray(target_weights, np.float64)
    twnz = (tw != 0).astype(np.float64)
    num = ((2.0 * P - sqs) * twnz).sum(axis=0)
    den = np.maximum((P * twnz).sum(axis=0), 1.0)
    giou_joint = num / den
    mse = 0.5 * (tw**2 * ssd).sum(axis=0) / (B * P)
    return np.float32(np.sum(mse + giou_joint) / K)


def kernel(output, target, target_weights):
    output = np.ascontiguousarray(np.asarray(output), dtype=np.float32)
    target = np.ascontiguousarray(np.asarray(target), dtype=np.float32)
    nc = _get_nc()
    in_maps = [
        {
            "output": output[i * B_LOC : (i + 1) * B_LOC],
            "target": target[i * B_LOC : (i + 1) * B_LOC],
        }
        for i in range(N_CORES)
    ]
    res = run_bass_kernel_spmd(nc, in_maps, list(range(N_CORES)))
    parts = np.stack([res.results[i]["partials"] for i in range(N_CORES)])
    return np.asarray(_combine(parts, target_weights), dtype=np.float32)
